# revision 48
# baseline (speedup 1.0000x reference)
"""MatchingNet forward on 8 Trainium2 NeuronCores (Bass/Tile).

Math (reference):
    s_emb = l2norm(support @ W + b)   [Ns, E]
    q_emb = l2norm(query @ W + b)     [Nq, E]
    sims  = q_emb @ s_emb.T           [Nq, Ns]
    preds = softmax(sims, axis=1) @ one_hot(labels, C)   [Nq, C]

Sharding: query rows are data-parallel (1024 per core). The support
encode is also sharded (512 rows per core) and the normalized support
embeddings are AllGathered on-chip in 128-row chunks (so sims starts on
the first chunk while later ones are in flight), which halves the
per-core FLOPs vs replicating the support encode on every core.

Device layout: embeddings are computed TRANSPOSED ([emb, n] with emb on
partitions) so the whole chain needs no transposes:
    s_embT tile = W_chunk.T @ supportT_chunk   (lhsT = W as stored)
    simsT  tile = s_normT_chunk.T @ q_normT    ([sup, q] layout)
    preds       = exp_simsT_chunk.T @ one_hot_aug  ([q, C+1] layout)
one_hot is augmented with a ones column so the softmax denominator
falls out of the same matmul; division happens per query partition.
Cosine sims are in [-1, 1] so softmax needs no max subtraction.

All device inputs are pre-laid-out on the host so every input DMA is a
contiguous copy. Matmul inputs are bf16 (fp32 PSUM accumulation);
error << the 2e-2 gate.

Scheduling notes (hard-won):
 - nothing that waits on the collective may sit ahead of other work
   in any engine's instruction stream (the engine would block on the
   collective semaphore and kill the overlap) -- so the gather-read
   DMAs are issued AFTER both encodes, all on the gpsimd queue (no
   later compute there), as 16 core-pair reads in consumption order:
   16 total keeps Tile's DMA-completion watermarks exact per read, so
   each sims tile unblocks the moment its own read lands;
 - input DMA order: W tiles stream on the scalar queue while supX
   streams on sync (first tiles split in half), so the first matmul
   starts ~12us in and the encoder m-loop never waits on a W tile;
 - the ones-matmuls (norm partition-reduction) are deferred behind each
   block's main matmuls so the PE never waits mid-stream on the
   ACT->DVE square chain;
 - preds accumulate in sims-chunk PRODUCTION order with all 8 query
   tiles in flight (packed two per PSUM bank; only the first matmul in
   a bank uses start=True -- the others land on has_written-clear
   elements and overwrite), one preds matmul interleaved after each
   sims matmul (its weight load hides under the 512-wide DoubleRow
   stream), so only ~8 tiny matmuls trail the last sims chunk;
 - the final divisions use dedicated buffers (no reuse waits) and
   alternate DVE/ACT so the output tail is a few us, overlapping the
   per-tile output DMAs.
"""

import numpy as np
import ml_dtypes

import concourse.bacc as bacc
import concourse.mybir as mybir
import concourse.tile as tile
from concourse.bass_utils import run_bass_kernel_spmd

F32 = mybir.dt.float32
BF16 = mybir.dt.bfloat16
FP8 = mybir.dt.float8e4
# normalized embeddings are scaled by 16 before the fp8 cast (values land in
# e4m3's normal range); the sims matmul result is scaled back inside exp().
# W is scaled by 32 for the same reason; the encoder bias-add scales back.
EMB_SCALE = 16.0
W_SCALE = 32.0
AF = mybir.ActivationFunctionType

# Full-problem config (hardcoded; the grading harness provides exactly these)
N_SUPPORT = 4096
N_QUERY = 8192
IN_DIM = 2048
EMB_DIM = 1024
N_CLS = 64
N_CORES = 8
NQ_SHARD = N_QUERY // N_CORES  # 1024 query rows per core


def build_nc(NS, NQ, IN, EMB, NCLS, n_cores=N_CORES):
    """Per-core Bass program. NCLS includes the +1 ones column.

    NS is the GLOBAL support count; each core encodes NS/n_cores rows
    and AllGathers the normalized embeddings in 128-row chunks.
    """
    KCH = IN // 128    # contraction chunks for the encoder matmul
    MCH = EMB // 128   # emb chunks (partition blocks of the embT layout)
    SCH = NS // 128    # support chunks
    NS_SH = NS // n_cores
    NB_Q = NQ // 512
    SBS = 256          # support encode block size
    CW = 128           # AllGather chunk width (support rows per chunk)
    G = NS_SH // CW    # chunks per core
    SPB = SBS // CW    # chunks shipped per support block
    assert NS % 512 == 0 and NQ % 512 == 0 and IN % 128 == 0 and EMB % 128 == 0
    assert NS_SH % SBS == 0

    nc = bacc.Bacc()
    # host-pre-laid-out inputs (see _prep_inputs): every DMA is contiguous
    supX = nc.declare_dram_parameter("supX", [NS_SH // SBS, 128, KCH, SBS],
                                     FP8, isOutput=False)
    qX = nc.declare_dram_parameter("qX", [NB_Q, 128, KCH, 512], FP8,
                                   isOutput=False)
    Wd = nc.declare_dram_parameter("W", [MCH, 128, KCH, 128], FP8,
                                   isOutput=False)
    bd = nc.declare_dram_parameter("b", [128, MCH], F32, isOutput=False)
    ohd = nc.declare_dram_parameter("onehot", [128, SCH, NCLS], BF16,
                                    isOutput=False)
    outd = nc.declare_dram_parameter("out", [NQ, NCLS - 1], F32, isOutput=True)

    with tile.TileContext(nc) as tc:
        with (
            tc.tile_pool(name="singles", bufs=1) as singles,
            tc.tile_pool(name="emb_pool", bufs=1) as emb_pool,
            tc.tile_pool(name="small", bufs=8) as small,
            tc.tile_pool(name="ps_mm", bufs=4, space="PSUM") as ps_mm,
        ):
            b_sb = singles.tile([128, MCH], F32)
            nc.sync.dma_start(out=b_sb, in_=bd[:, :])
            ones_sb = singles.tile([128, 1], BF16)
            nc.vector.memset(ones_sb, 1.0)
            # one_hot_aug chunks on the scalar queue, after W and qX
            # (nothing needs onehot until preds)
            oh_sb = singles.tile([128, SCH, NCLS], BF16)

            # resident normalized embeddings, transposed ([emb, n], fp8)
            q_nrm = emb_pool.tile([128, MCH, NQ], FP8, name="q_nrm", tag="q_nrm")
            # gathered support embeddings: per (chunk, core-group) tiles,
            # each written by one gather-read DMA. <=16 reads total keeps
            # Tile's DMA-completion tracking EXACT per read (at 17+ the
            # watermarks coalesce and the first consumers stall 7-12us;
            # 2 fused reads made the first tile wait a whole 512KB batch).
            # Chunk 0 leads with single-core reads (its first read gates
            # the whole sims start); late chunks use 4-core reads to stay
            # inside the 16-read budget: 5+4+2+2 = 13.
            assert G == 4
            RGS = [[1, 1, 2, 2, 2], [2, 2, 2, 2], [4, 4], [4, 4]]
            gt = [[emb_pool.tile([128, rg, MCH, CW], FP8,
                                 name=f"gt{g}_{j}", tag=f"gt{g}_{j}")
                   for j, rg in enumerate(RGS[g])] for g in range(G)]

            def gt_tile(g, c):
                c0 = 0
                for j, rg in enumerate(RGS[g]):
                    if c < c0 + rg:
                        return gt[g][j][:, c - c0]
                    c0 += rg
                raise AssertionError

            with (
                tc.tile_pool(name="w_pool", bufs=1) as w_pool,
                tc.tile_pool(name="xin", bufs=5) as xin,
                tc.tile_pool(name="pre_pool", bufs=3) as pre_pool,
                tc.tile_pool(name="sq_pool", bufs=2) as sq_pool,
                tc.tile_pool(name="bc_pool", bufs=2) as bc_pool,
                tc.tile_pool(name="loc_pool", bufs=1) as loc_pool,
                tc.tile_pool(name="dscr", bufs=2, space="DRAM") as dscr,
                tc.tile_pool(name="cc_pool", bufs=1, space="DRAM") as cc_pool,
                tc.tile_pool(name="ps_n2", bufs=2, space="PSUM") as ps_n2,
            ):
                # DMA schedule: sync carries supX (first block split in two
                # k-halves so matmuls start after ~0.4MB); scalar carries W
                # (W0 also split), then qX, then onehot. The encoder m-loop
                # consumes W[m] long after it lands.
                W_sb = [w_pool.tile([128, KCH, 128], FP8, tag=f"w{m}",
                                    name=f"w{m}") for m in range(MCH)]
                KH = KCH // 2
                nc.scalar.dma_start(out=W_sb[0][:, :KH, :], in_=Wd[0][:, :KH, :])
                sup_xks, q_xks = [], []
                t0 = xin.tile([128, KCH, SBS], FP8, tag="xk", name="xk")
                nc.sync.dma_start(out=t0[:, :KH, :], in_=supX[0][:, :KH, :])
                nc.sync.dma_start(out=t0[:, KH:, :], in_=supX[0][:, KH:, :])
                sup_xks.append(t0)
                nc.scalar.dma_start(out=W_sb[0][:, KH:, :], in_=Wd[0][:, KH:, :])
                for m in range(1, MCH):
                    nc.scalar.dma_start(out=W_sb[m], in_=Wd[m])
                for nb in range(1, NS_SH // SBS):
                    t = xin.tile([128, KCH, SBS], FP8, tag="xk", name="xk")
                    nc.sync.dma_start(out=t, in_=supX[nb])
                    sup_xks.append(t)
                for nb in range(NB_Q):
                    t = xin.tile([128, KCH, 512], FP8, tag="xk", name="xk")
                    nc.scalar.dma_start(out=t, in_=qX[nb])
                    q_xks.append(t)
                nc.scalar.dma_start(out=oh_sb, in_=ohd[:, :, :])

                # local normalized support shard, CHUNK-major so each
                # AllGather ship copy is one contiguous DMA
                s_loc = loc_pool.tile([128, G, MCH, CW], FP8, name="s_loc")
                ag_outs = []

                def ship_chunk(g):
                    ag_in = cc_pool.tile([128, MCH * CW], FP8,
                                         name=f"ag_in{g}", tag=f"ag_in{g}")
                    nc.sync.dma_start(
                        out=ag_in.rearrange("p (m v) -> p m v", m=MCH),
                        in_=s_loc[:, g])
                    ag_out = cc_pool.tile(
                        [n_cores * 128, MCH * CW], FP8, name=f"ag_out{g}",
                        tag=f"ag_out{g}", addr_space="Shared")
                    nc.gpsimd.collective_compute(
                        "AllGather",
                        mybir.AluOpType.bypass,
                        replica_groups=[list(range(n_cores))],
                        ins=[ag_in],
                        outs=[ag_out],
                    )
                    ag_outs.append(ag_out)

                def encode(NB, bs, xks, to_chunks=False, block_done=None,
                           nb_lo=0):
                    """l2norm(x @ W + b).T, emb-chunked; writes q_nrm or
                    the chunk-major s_loc."""
                    for nb in range(nb_lo, NB):
                        xk = xks[nb]
                        n2 = ps_n2.tile([1, bs], F32, tag="n2", name="n2")
                        pre = pre_pool.tile([128, MCH, bs], BF16, tag="pre",
                                            name="pre")
                        sq = sq_pool.tile([128, MCH, bs], BF16, tag="sq",
                                          name="sq")
                        assert KCH % 2 == 0
                        for m in range(MCH):
                            ps = ps_mm.tile([128, bs], F32, tag="mmps", name="ps")
                            for t in range(KCH // 2):
                                nc.tensor.matmul(
                                    ps,
                                    lhsT=W_sb[m][:, 2 * t:2 * t + 2, :],
                                    rhs=xk[:, 2 * t:2 * t + 2, :],
                                    start=(t == 0),
                                    stop=(t == KCH // 2 - 1),
                                    perf_mode=mybir.MatmulPerfMode.DoubleRow,
                                )
                            # bias add (rescaling the fp8 W) + PSUM->SBUF bf16
                            nc.scalar.activation(pre[:, m, :], ps, AF.Identity,
                                                 bias=b_sb[:, m:m + 1],
                                                 scale=1.0 / W_SCALE)
                            nc.vector.tensor_mul(
                                sq[:, m, :], pre[:, m, :], pre[:, m, :])
                            # column-sums of squares via ones-matmuls
                            # (partition reduce), trailing one main group so
                            # the PE never waits mid-stream on the ACT->DVE
                            # square chain
                            if m >= 1:
                                nc.tensor.matmul(
                                    n2, lhsT=ones_sb, rhs=sq[:, m - 1, :],
                                    start=(m == 1), stop=False,
                                )
                        nc.tensor.matmul(
                            n2, lhsT=ones_sb, rhs=sq[:, MCH - 1, :],
                            start=False, stop=True,
                        )
                        nrm = small.tile([1, bs], F32, tag="nrm", name="nrm")
                        nc.scalar.activation(nrm, n2, AF.Sqrt,
                                             scale=1.0 / (EMB_SCALE * EMB_SCALE))
                        inv = small.tile([1, bs], F32, tag="inv", name="inv")
                        nc.vector.reciprocal(inv, nrm)
                        # partition-broadcast inv: SBUF[1,bs] -> DRAM -> SBUF
                        # (DMA only allows a zero partition step on DRAM sources)
                        iscr = dscr.tile([1, bs], F32, tag="iscr", name="iscr")
                        nc.sync.dma_start(out=iscr, in_=inv)
                        invb = bc_pool.tile([128, bs], F32, tag="invb",
                                            name="invb")
                        nc.sync.dma_start(out=invb,
                                          in_=iscr.partition_broadcast(128))
                        for m in range(MCH):
                            if to_chunks:
                                for h in range(SPB):
                                    hs = slice(h * CW, (h + 1) * CW)
                                    nc.vector.tensor_mul(
                                        s_loc[:, nb * SPB + h, m, :],
                                        pre[:, m, hs], invb[:, hs])
                            else:
                                vs = slice(nb * bs, (nb + 1) * bs)
                                nc.vector.tensor_mul(q_nrm[:, m, vs],
                                                     pre[:, m, :], invb)
                        if block_done is not None:
                            block_done(nb)

                def sup_done(nb):
                    for h in range(SPB):
                        ship_chunk(nb * SPB + h)

                # ALL support blocks first: an AllGather only starts once
                # the LAST core (cores start tens of us apart) rings its
                # doorbell, and chunk doorbells ring at normalize time --
                # the query encode then overlaps the gather latency.
                nsb = NS_SH // SBS
                for blk in range(nsb):
                    encode(blk + 1, SBS, sup_xks, to_chunks=True,
                           block_done=sup_done, nb_lo=blk)
                for blk in range(NB_Q):
                    encode(blk + 1, 512, q_xks, nb_lo=blk)

                # gather-read DMAs AFTER all encode work (they wait on the
                # collective semaphore; anything queued behind them would
                # stall), all on the gpsimd queue (no later compute there;
                # the sync queue measured 3-6x slower on these reads), in
                # consumption order so tiles unblock as each read lands.
                for g in range(G):
                    src = ag_outs[g].rearrange("(c p) (m v) -> p c m v",
                                               p=128, m=MCH)
                    c0 = 0
                    for j, rg in enumerate(RGS[g]):
                        nc.gpsimd.dma_start(out=gt[g][j],
                                            in_=src[:, c0:c0 + rg])
                        c0 += rg

            with (
                tc.tile_pool(name="exp_pool", bufs=1) as exp_pool,
                tc.tile_pool(name="ps_pred", bufs=1, space="PSUM") as ps_pred,
                tc.tile_pool(name="outp", bufs=8) as outp,
            ):
                # exp(simsT) in [sup, q] layout, bf16, sup-chunked.
                # Iterate gather-chunk-major so each AllGather chunk is
                # consumed as soon as it lands, cores in read order.
                # Global support chunk index of (g, c) is c*G + g
                # (core-major support order).
                work = [(c * G + g, gt_tile(g, c))
                        for g in range(G) for c in range(n_cores)]
                expT = exp_pool.tile([128, SCH, NQ], BF16)
                assert MCH % 2 == 0
                NWORK = len(work)
                # preds accumulators: 8 query tiles packed two per PSUM
                # bank, all accumulating in sims production order. Only
                # the first matmul into each bank uses start=True (clears
                # the whole bank's has_written bits); its bank-mate's
                # first matmul lands on cleared bits and overwrites.
                NQB = NQ // 128
                pp = [ps_pred.tile([128, 2, NCLS], F32, tag=f"pp{j}",
                                   name=f"pp{j}") for j in range(NQB // 2)]

                def emit_pred(j, sb, qb):
                    nc.tensor.matmul(
                        pp[qb // 2][:, qb % 2, :],
                        lhsT=expT[:, sb, qb * 128:(qb + 1) * 128],
                        rhs=oh_sb[:, sb, :],
                        start=(j == 0 and qb % 2 == 0),
                        stop=(j == NWORK - 1 and qb % 2 == 1),
                    )

                for idx, (sb, src_tile) in enumerate(work):
                    ps2 = [ps_mm.tile([128, 512], F32, tag="mmps", name="ps")
                           for _ in range(2)]
                    for t in range(MCH // 2):
                        for qh in range(2):
                            nc.tensor.matmul(
                                ps2[qh],
                                lhsT=src_tile[:, 2 * t:2 * t + 2, :],
                                rhs=q_nrm[:, 2 * t:2 * t + 2,
                                          qh * 512:(qh + 1) * 512],
                                start=(t == 0),
                                stop=(t == MCH // 2 - 1),
                                perf_mode=mybir.MatmulPerfMode.DoubleRow,
                            )
                            # preds for the PREVIOUS chunk, one per sims
                            # matmul: the tiny 128-col weight load hides
                            # under the 512-wide DoubleRow matmul and the
                            # exp ACT had a full chunk of matmuls to finish
                            if idx >= 1:
                                emit_pred(idx - 1, work[idx - 1][0],
                                          2 * t + qh)
                    for qh in range(2):
                        nc.scalar.activation(
                            expT[:, sb, qh * 512:(qh + 1) * 512], ps2[qh],
                            AF.Exp, scale=1.0 / (EMB_SCALE * EMB_SCALE))
                for qb in range(NQB):
                    emit_pred(NWORK - 1, work[NWORK - 1][0], qb)

                # softmax division: the ones column is the denominator.
                # Dedicated buffers (no reuse waits), multiplies alternate
                # DVE/ACT (two parallel pipes), and the outputs ship as TWO
                # fused DMAs on different queues -- eight separate
                # dma_starts cost ~600ns of descriptor-gen EACH, serial on
                # one engine, which dominated the old tail.
                oview = outd.rearrange("(qb p) c -> p qb c", p=128)
                ot_all = outp.tile([128, NQB, NCLS - 1], F32, name="ot_all")
                for qb in range(NQB):
                    acc = pp[qb // 2][:, qb % 2, :]
                    rec = small.tile([128, 1], F32, tag="rec", name="rec")
                    nc.vector.reciprocal(rec, acc[:, NCLS - 1:NCLS])
                    if qb % 2 == 0:
                        nc.vector.tensor_scalar_mul(
                            ot_all[:, qb, :], acc[:, 0:NCLS - 1], rec)
                    else:
                        nc.scalar.activation(ot_all[:, qb, :],
                                             acc[:, 0:NCLS - 1],
                                             AF.Copy, scale=rec)
                    if qb == NQB // 2 - 1:
                        nc.sync.dma_start(out=oview[:, :NQB // 2],
                                          in_=ot_all[:, :NQB // 2])
                nc.scalar.dma_start(out=oview[:, NQB // 2:],
                                    in_=ot_all[:, NQB // 2:])
    nc.finalize()
    return nc


_NC_CACHE = {}


def _get_nc(key):
    if key not in _NC_CACHE:
        NS, NQ, IN, EMB, NCLS = key
        _NC_CACHE[key] = build_nc(NS, NQ, IN, EMB, NCLS)
    return _NC_CACHE[key]


def _x_layout(x, kch, bs=512):
    """[NV, IN] fp32 -> [NV/bs, 128, KCH, bs] fp8 so each bs-row encoder
    block is one contiguous DMA: H[nb,p,k,v] = x[nb*bs+v, k*128+p]."""
    nv, in_dim = x.shape
    h = x.reshape(nv // bs, bs, kch, 128).transpose(0, 3, 2, 1)
    return np.ascontiguousarray(h.astype(ml_dtypes.float8_e4m3))


def _prep_inputs(support, query, W, b, support_labels, num_classes, n_cores):
    ncls = int(num_classes)
    bf = ml_dtypes.bfloat16
    support = np.asarray(support, np.float32)
    query = np.asarray(query, np.float32)
    W = np.asarray(W, np.float32)
    in_dim, emb = W.shape
    kch, mch = in_dim // 128, emb // 128
    ns = support.shape[0]
    # W[m, p, k, e] = W_SCALE * W[k*128+p, m*128+e]
    Wh = np.ascontiguousarray(
        (W * W_SCALE).reshape(kch, 128, mch, 128)
        .transpose(2, 1, 0, 3).astype(ml_dtypes.float8_e4m3))
    # b[p, m] = b[m*128+p]
    bh = np.ascontiguousarray(np.asarray(b, np.float32).reshape(mch, 128).T)
    labels = np.asarray(support_labels).astype(np.int64)
    oh = np.zeros((ns, ncls + 1), dtype=bf)
    oh[np.arange(ns), labels] = 1
    oh[:, ncls] = 1  # ones column -> softmax denominator
    # oh[p, c, h] = onehot[c*128+p, h]
    ohh = np.ascontiguousarray(
        oh.reshape(ns // 128, 128, ncls + 1).transpose(1, 0, 2))
    nq_shard = query.shape[0] // n_cores
    ns_shard = ns // n_cores
    qh_all = _x_layout(query, kch)  # [NQ/512, 128, KCH, 512]
    nbq = nq_shard // 512
    in_maps = []
    for i in range(n_cores):
        sup_i = support[i * ns_shard:(i + 1) * ns_shard]
        in_maps.append({
            "supX": _x_layout(sup_i, kch, 256),
            "qX": np.ascontiguousarray(qh_all[i * nbq:(i + 1) * nbq]),
            "W": Wh,
            "b": bh,
            "onehot": ohh,
        })
    return in_maps


def _run(support, query, W, b, support_labels, num_classes, trace=False):
    ncls = int(num_classes)
    key = (support.shape[0], query.shape[0] // N_CORES, support.shape[1],
           W.shape[1], ncls + 1)
    nc = _get_nc(key)
    in_maps = _prep_inputs(support, query, W, b, support_labels, ncls, N_CORES)
    res = run_bass_kernel_spmd(nc, in_maps, list(range(N_CORES)), trace=trace)
    out = np.concatenate([r["out"] for r in res.results], axis=0)
    return out.astype(np.float32), res


def kernel(support, query, W, b, support_labels, num_classes):
    out, _ = _run(support, query, W, b, support_labels, num_classes, trace=False)
    return out


# revision 50
# speedup vs baseline: 1.0470x; 1.0470x over previous
"""MatchingNet forward on 8 Trainium2 NeuronCores (Bass/Tile).

Math (reference):
    s_emb = l2norm(support @ W + b)   [Ns, E]
    q_emb = l2norm(query @ W + b)     [Nq, E]
    sims  = q_emb @ s_emb.T           [Nq, Ns]
    preds = softmax(sims, axis=1) @ one_hot(labels, C)   [Nq, C]

Sharding: query rows are data-parallel (1024 per core). The support
encode is also sharded (512 rows per core) and the normalized support
embeddings are AllGathered on-chip in 128-row chunks (so sims starts on
the first chunk while later ones are in flight), which halves the
per-core FLOPs vs replicating the support encode on every core.

Device layout: embeddings are computed TRANSPOSED ([emb, n] with emb on
partitions) so the whole chain needs no transposes:
    s_embT tile = W_chunk.T @ supportT_chunk   (lhsT = W as stored)
    simsT  tile = s_normT_chunk.T @ q_normT    ([sup, q] layout)
    preds       = exp_simsT_chunk.T @ one_hot_aug  ([q, C+1] layout)
one_hot is augmented with a ones column so the softmax denominator
falls out of the same matmul; division happens per query partition.
Cosine sims are in [-1, 1] so softmax needs no max subtraction.

All device inputs are pre-laid-out on the host so every input DMA is a
contiguous copy. Matmul inputs are bf16 (fp32 PSUM accumulation);
error << the 2e-2 gate.

Scheduling notes (hard-won):
 - nothing that waits on the collective may sit ahead of other work
   in any engine's instruction stream (the engine would block on the
   collective semaphore and kill the overlap) -- so the gather-read
   DMAs are issued AFTER both encodes, all on the gpsimd queue (no
   later compute there), as 16 core-pair reads in consumption order:
   16 total keeps Tile's DMA-completion watermarks exact per read, so
   each sims tile unblocks the moment its own read lands;
 - input DMA order: W tiles stream on the scalar queue while supX
   streams on sync (first tiles split in half), so the first matmul
   starts ~12us in and the encoder m-loop never waits on a W tile;
 - the ones-matmuls (norm partition-reduction) are deferred behind each
   block's main matmuls so the PE never waits mid-stream on the
   ACT->DVE square chain;
 - preds accumulate in sims-chunk PRODUCTION order with all 8 query
   tiles in flight (packed two per PSUM bank; only the first matmul in
   a bank uses start=True -- the others land on has_written-clear
   elements and overwrite), one preds matmul interleaved after each
   sims matmul (its weight load hides under the 512-wide DoubleRow
   stream), so only ~8 tiny matmuls trail the last sims chunk;
 - the final divisions use dedicated buffers (no reuse waits) and
   alternate DVE/ACT so the output tail is a few us, overlapping the
   per-tile output DMAs.
"""

import numpy as np
import ml_dtypes

import concourse.bacc as bacc
import concourse.mybir as mybir
import concourse.tile as tile
from concourse.bass_utils import run_bass_kernel_spmd

F32 = mybir.dt.float32
BF16 = mybir.dt.bfloat16
FP8 = mybir.dt.float8e4
# normalized embeddings are scaled by 16 before the fp8 cast (values land in
# e4m3's normal range); the sims matmul result is scaled back inside exp().
# W is scaled by 32 for the same reason; the encoder bias-add scales back.
EMB_SCALE = 16.0
W_SCALE = 32.0
AF = mybir.ActivationFunctionType

# Full-problem config (hardcoded; the grading harness provides exactly these)
N_SUPPORT = 4096
N_QUERY = 8192
IN_DIM = 2048
EMB_DIM = 1024
N_CLS = 64
N_CORES = 8
NQ_SHARD = N_QUERY // N_CORES  # 1024 query rows per core


def build_nc(NS, NQ, IN, EMB, NCLS, n_cores=N_CORES):
    """Per-core Bass program. NCLS includes the +1 ones column.

    NS is the GLOBAL support count; each core encodes NS/n_cores rows
    and AllGathers the normalized embeddings in 128-row chunks.
    """
    KCH = IN // 128    # contraction chunks for the encoder matmul
    MCH = EMB // 128   # emb chunks (partition blocks of the embT layout)
    SCH = NS // 128    # support chunks
    NS_SH = NS // n_cores
    NB_Q = NQ // 512
    SBS = 256          # support encode block size
    CW = 128           # AllGather chunk width (support rows per chunk)
    G = NS_SH // CW    # chunks per core
    SPB = SBS // CW    # chunks shipped per support block
    assert NS % 512 == 0 and NQ % 512 == 0 and IN % 128 == 0 and EMB % 128 == 0
    assert NS_SH % SBS == 0

    nc = bacc.Bacc()
    # host-pre-laid-out inputs (see _prep_inputs): every DMA is contiguous
    supX = nc.declare_dram_parameter("supX", [NS_SH // SBS, 128, KCH, SBS],
                                     FP8, isOutput=False)
    qX = nc.declare_dram_parameter("qX", [NB_Q, 128, KCH, 512], FP8,
                                   isOutput=False)
    Wd = nc.declare_dram_parameter("W", [MCH, 128, KCH, 128], FP8,
                                   isOutput=False)
    bd = nc.declare_dram_parameter("b", [128, MCH], F32, isOutput=False)
    ohd = nc.declare_dram_parameter("onehot", [128, SCH, NCLS], BF16,
                                    isOutput=False)
    outd = nc.declare_dram_parameter("out", [NQ, NCLS - 1], F32, isOutput=True)

    with tile.TileContext(nc) as tc:
        with (
            tc.tile_pool(name="singles", bufs=1) as singles,
            tc.tile_pool(name="emb_pool", bufs=1) as emb_pool,
            tc.tile_pool(name="small", bufs=8) as small,
            tc.tile_pool(name="ps_mm", bufs=4, space="PSUM") as ps_mm,
        ):
            b_sb = singles.tile([128, MCH], F32)
            nc.sync.dma_start(out=b_sb, in_=bd[:, :])
            ones_sb = singles.tile([128, 1], BF16)
            nc.vector.memset(ones_sb, 1.0)
            # one_hot_aug chunks on the scalar queue, after W and qX
            # (nothing needs onehot until preds)
            oh_sb = singles.tile([128, SCH, NCLS], BF16)

            # resident normalized embeddings, transposed ([emb, n], fp8)
            q_nrm = emb_pool.tile([128, MCH, NQ], FP8, name="q_nrm", tag="q_nrm")
            # gathered support embeddings: per (chunk, core-pair) tiles,
            # each written by one gather-read DMA. 16 reads total keeps
            # Tile's DMA-completion tracking EXACT per read (at 32 reads
            # the watermarks coalesce and the first consumers stall ~7us;
            # 2 fused reads made the first tile wait a whole 512KB batch).
            NPR = n_cores // 2  # core-pairs per chunk
            gt = [[emb_pool.tile([128, 2, MCH, CW], FP8,
                                 name=f"gt{g}_{pr}", tag=f"gt{g}_{pr}")
                   for pr in range(NPR)] for g in range(G)]

            with (
                tc.tile_pool(name="w_pool", bufs=1) as w_pool,
                tc.tile_pool(name="xin", bufs=5) as xin,
                tc.tile_pool(name="pre_pool", bufs=3) as pre_pool,
                tc.tile_pool(name="sq_pool", bufs=2) as sq_pool,
                tc.tile_pool(name="bc_pool", bufs=2) as bc_pool,
                tc.tile_pool(name="loc_pool", bufs=1) as loc_pool,
                tc.tile_pool(name="dscr", bufs=2, space="DRAM") as dscr,
                tc.tile_pool(name="cc_pool", bufs=1, space="DRAM") as cc_pool,
                tc.tile_pool(name="ps_n2", bufs=2, space="PSUM") as ps_n2,
            ):
                # DMA schedule: sync carries supX (first block split in two
                # k-halves so matmuls start after ~0.4MB); scalar carries W
                # (W0 also split), then qX, then onehot. The encoder m-loop
                # consumes W[m] long after it lands.
                W_sb = [w_pool.tile([128, KCH, 128], FP8, tag=f"w{m}",
                                    name=f"w{m}") for m in range(MCH)]
                KH = KCH // 2
                nc.scalar.dma_start(out=W_sb[0][:, :KH, :], in_=Wd[0][:, :KH, :])
                sup_xks, q_xks = [], []
                t0 = xin.tile([128, KCH, SBS], FP8, tag="xk", name="xk")
                nc.sync.dma_start(out=t0[:, :KH, :], in_=supX[0][:, :KH, :])
                nc.sync.dma_start(out=t0[:, KH:, :], in_=supX[0][:, KH:, :])
                sup_xks.append(t0)
                nc.scalar.dma_start(out=W_sb[0][:, KH:, :], in_=Wd[0][:, KH:, :])
                for m in range(1, MCH):
                    nc.scalar.dma_start(out=W_sb[m], in_=Wd[m])
                for nb in range(1, NS_SH // SBS):
                    t = xin.tile([128, KCH, SBS], FP8, tag="xk", name="xk")
                    nc.sync.dma_start(out=t, in_=supX[nb])
                    sup_xks.append(t)
                for nb in range(NB_Q):
                    t = xin.tile([128, KCH, 512], FP8, tag="xk", name="xk")
                    nc.scalar.dma_start(out=t, in_=qX[nb])
                    q_xks.append(t)
                nc.scalar.dma_start(out=oh_sb, in_=ohd[:, :, :])

                # local normalized support shard, CHUNK-major so each
                # AllGather ship copy is one contiguous DMA
                s_loc = loc_pool.tile([128, G, MCH, CW], FP8, name="s_loc")
                ag_outs = []

                def ship_chunk(g):
                    ag_in = cc_pool.tile([128, MCH * CW], FP8,
                                         name=f"ag_in{g}", tag=f"ag_in{g}")
                    nc.sync.dma_start(
                        out=ag_in.rearrange("p (m v) -> p m v", m=MCH),
                        in_=s_loc[:, g])
                    ag_out = cc_pool.tile(
                        [n_cores * 128, MCH * CW], FP8, name=f"ag_out{g}",
                        tag=f"ag_out{g}", addr_space="Shared")
                    nc.gpsimd.collective_compute(
                        "AllGather",
                        mybir.AluOpType.bypass,
                        replica_groups=[list(range(n_cores))],
                        ins=[ag_in],
                        outs=[ag_out],
                    )
                    ag_outs.append(ag_out)

                def encode(NB, bs, xks, to_chunks=False, block_done=None,
                           nb_lo=0):
                    """l2norm(x @ W + b).T, emb-chunked; writes q_nrm or
                    the chunk-major s_loc."""
                    for nb in range(nb_lo, NB):
                        xk = xks[nb]
                        n2 = ps_n2.tile([1, bs], F32, tag="n2", name="n2")
                        pre = pre_pool.tile([128, MCH, bs], BF16, tag="pre",
                                            name="pre")
                        sq = sq_pool.tile([128, MCH, bs], BF16, tag="sq",
                                          name="sq")
                        assert KCH % 2 == 0
                        for m in range(MCH):
                            ps = ps_mm.tile([128, bs], F32, tag="mmps", name="ps")
                            for t in range(KCH // 2):
                                nc.tensor.matmul(
                                    ps,
                                    lhsT=W_sb[m][:, 2 * t:2 * t + 2, :],
                                    rhs=xk[:, 2 * t:2 * t + 2, :],
                                    start=(t == 0),
                                    stop=(t == KCH // 2 - 1),
                                    perf_mode=mybir.MatmulPerfMode.DoubleRow,
                                )
                            # bias add (rescaling the fp8 W) + PSUM->SBUF bf16
                            nc.scalar.activation(pre[:, m, :], ps, AF.Identity,
                                                 bias=b_sb[:, m:m + 1],
                                                 scale=1.0 / W_SCALE)
                            nc.vector.tensor_mul(
                                sq[:, m, :], pre[:, m, :], pre[:, m, :])
                            # column-sums of squares via ones-matmuls
                            # (partition reduce), trailing one main group so
                            # the PE never waits mid-stream on the ACT->DVE
                            # square chain
                            if m >= 1:
                                nc.tensor.matmul(
                                    n2, lhsT=ones_sb, rhs=sq[:, m - 1, :],
                                    start=(m == 1), stop=False,
                                )
                        nc.tensor.matmul(
                            n2, lhsT=ones_sb, rhs=sq[:, MCH - 1, :],
                            start=False, stop=True,
                        )
                        nrm = small.tile([1, bs], F32, tag="nrm", name="nrm")
                        nc.scalar.activation(nrm, n2, AF.Sqrt,
                                             scale=1.0 / (EMB_SCALE * EMB_SCALE))
                        inv = small.tile([1, bs], F32, tag="inv", name="inv")
                        nc.vector.reciprocal(inv, nrm)
                        # partition-broadcast inv: SBUF[1,bs] -> DRAM -> SBUF
                        # (DMA only allows a zero partition step on DRAM sources)
                        iscr = dscr.tile([1, bs], F32, tag="iscr", name="iscr")
                        nc.sync.dma_start(out=iscr, in_=inv)
                        invb = bc_pool.tile([128, bs], F32, tag="invb",
                                            name="invb")
                        nc.sync.dma_start(out=invb,
                                          in_=iscr.partition_broadcast(128))
                        for m in range(MCH):
                            if to_chunks:
                                for h in range(SPB):
                                    hs = slice(h * CW, (h + 1) * CW)
                                    nc.vector.tensor_mul(
                                        s_loc[:, nb * SPB + h, m, :],
                                        pre[:, m, hs], invb[:, hs])
                            else:
                                vs = slice(nb * bs, (nb + 1) * bs)
                                nc.vector.tensor_mul(q_nrm[:, m, vs],
                                                     pre[:, m, :], invb)
                        if block_done is not None:
                            block_done(nb)

                def sup_done(nb):
                    for h in range(SPB):
                        ship_chunk(nb * SPB + h)

                # ALL support blocks first: an AllGather only starts once
                # the LAST core (cores start tens of us apart) rings its
                # doorbell, and chunk doorbells ring at normalize time --
                # the query encode then overlaps the gather latency.
                nsb = NS_SH // SBS
                for blk in range(nsb):
                    encode(blk + 1, SBS, sup_xks, to_chunks=True,
                           block_done=sup_done, nb_lo=blk)
                for blk in range(NB_Q):
                    encode(blk + 1, 512, q_xks, nb_lo=blk)

                # gather-read DMAs AFTER all encode work (they wait on the
                # collective semaphore; anything queued behind them would
                # stall), all on the gpsimd queue (no later compute there;
                # the sync queue measured 3-6x slower on these reads), in
                # consumption order so tiles unblock as each read lands.
                for g in range(G):
                    src = ag_outs[g].rearrange("(c p) (m v) -> p c m v",
                                               p=128, m=MCH)
                    for pr in range(NPR):
                        nc.gpsimd.dma_start(out=gt[g][pr],
                                            in_=src[:, 2 * pr:2 * pr + 2])

            with (
                tc.tile_pool(name="exp_pool", bufs=1) as exp_pool,
                tc.tile_pool(name="ps_pred", bufs=1, space="PSUM") as ps_pred,
                tc.tile_pool(name="outp", bufs=8) as outp,
            ):
                # exp(simsT) in [sup, q] layout, bf16, sup-chunked.
                # Iterate gather-chunk-major so each AllGather chunk is
                # consumed as soon as it lands, cores in read order.
                # Global support chunk index of (g, c) is c*G + g
                # (core-major support order).
                work = [(c * G + g, gt[g][c // 2][:, c % 2])
                        for g in range(G) for c in range(n_cores)]
                expT = exp_pool.tile([128, SCH, NQ], BF16)
                assert MCH % 2 == 0
                NWORK = len(work)
                # preds accumulators: 8 query tiles packed two per PSUM
                # bank, all accumulating in sims production order. Only
                # the first matmul into each bank uses start=True (clears
                # the whole bank's has_written bits); its bank-mate's
                # first matmul lands on cleared bits and overwrites.
                NQB = NQ // 128
                pp = [ps_pred.tile([128, 2, NCLS], F32, tag=f"pp{j}",
                                   name=f"pp{j}") for j in range(NQB // 2)]

                def emit_pred(j, sb, qb):
                    nc.tensor.matmul(
                        pp[qb // 2][:, qb % 2, :],
                        lhsT=expT[:, sb, qb * 128:(qb + 1) * 128],
                        rhs=oh_sb[:, sb, :],
                        start=(j == 0 and qb % 2 == 0),
                        stop=(j == NWORK - 1 and qb % 2 == 1),
                    )

                for idx, (sb, src_tile) in enumerate(work):
                    ps2 = [ps_mm.tile([128, 512], F32, tag="mmps", name="ps")
                           for _ in range(2)]
                    for t in range(MCH // 2):
                        for qh in range(2):
                            nc.tensor.matmul(
                                ps2[qh],
                                lhsT=src_tile[:, 2 * t:2 * t + 2, :],
                                rhs=q_nrm[:, 2 * t:2 * t + 2,
                                          qh * 512:(qh + 1) * 512],
                                start=(t == 0),
                                stop=(t == MCH // 2 - 1),
                                perf_mode=mybir.MatmulPerfMode.DoubleRow,
                            )
                            # preds for the PREVIOUS chunk, one per sims
                            # matmul: the tiny 128-col weight load hides
                            # under the 512-wide DoubleRow matmul and the
                            # exp ACT had a full chunk of matmuls to finish
                            if idx >= 1:
                                emit_pred(idx - 1, work[idx - 1][0],
                                          2 * t + qh)
                    for qh in range(2):
                        nc.scalar.activation(
                            expT[:, sb, qh * 512:(qh + 1) * 512], ps2[qh],
                            AF.Exp, scale=1.0 / (EMB_SCALE * EMB_SCALE))
                for qb in range(NQB):
                    emit_pred(NWORK - 1, work[NWORK - 1][0], qb)

                # softmax division: the ones column is the denominator.
                # Dedicated buffers (no reuse waits), multiplies alternate
                # DVE/ACT (two parallel pipes), and the outputs ship as TWO
                # fused DMAs on different queues -- eight separate
                # dma_starts cost ~600ns of descriptor-gen EACH, serial on
                # one engine, which dominated the old tail.
                oview = outd.rearrange("(qb p) c -> p qb c", p=128)
                ot_all = outp.tile([128, NQB, NCLS - 1], F32, name="ot_all")
                for qb in range(NQB):
                    acc = pp[qb // 2][:, qb % 2, :]
                    rec = small.tile([128, 1], F32, tag="rec", name="rec")
                    nc.vector.reciprocal(rec, acc[:, NCLS - 1:NCLS])
                    if qb % 2 == 0:
                        nc.vector.tensor_scalar_mul(
                            ot_all[:, qb, :], acc[:, 0:NCLS - 1], rec)
                    else:
                        nc.scalar.activation(ot_all[:, qb, :],
                                             acc[:, 0:NCLS - 1],
                                             AF.Copy, scale=rec)
                    if qb == NQB // 2 - 1:
                        nc.sync.dma_start(out=oview[:, :NQB // 2],
                                          in_=ot_all[:, :NQB // 2])
                nc.scalar.dma_start(out=oview[:, NQB // 2:],
                                    in_=ot_all[:, NQB // 2:])
    nc.finalize()
    return nc


_NC_CACHE = {}


def _get_nc(key):
    if key not in _NC_CACHE:
        NS, NQ, IN, EMB, NCLS = key
        _NC_CACHE[key] = build_nc(NS, NQ, IN, EMB, NCLS)
    return _NC_CACHE[key]


def _x_layout(x, kch, bs=512):
    """[NV, IN] fp32 -> [NV/bs, 128, KCH, bs] fp8 so each bs-row encoder
    block is one contiguous DMA: H[nb,p,k,v] = x[nb*bs+v, k*128+p]."""
    nv, in_dim = x.shape
    h = x.reshape(nv // bs, bs, kch, 128).transpose(0, 3, 2, 1)
    return np.ascontiguousarray(h.astype(ml_dtypes.float8_e4m3))


def _prep_inputs(support, query, W, b, support_labels, num_classes, n_cores):
    ncls = int(num_classes)
    bf = ml_dtypes.bfloat16
    support = np.asarray(support, np.float32)
    query = np.asarray(query, np.float32)
    W = np.asarray(W, np.float32)
    in_dim, emb = W.shape
    kch, mch = in_dim // 128, emb // 128
    ns = support.shape[0]
    # W[m, p, k, e] = W_SCALE * W[k*128+p, m*128+e]
    Wh = np.ascontiguousarray(
        (W * W_SCALE).reshape(kch, 128, mch, 128)
        .transpose(2, 1, 0, 3).astype(ml_dtypes.float8_e4m3))
    # b[p, m] = b[m*128+p]
    bh = np.ascontiguousarray(np.asarray(b, np.float32).reshape(mch, 128).T)
    labels = np.asarray(support_labels).astype(np.int64)
    oh = np.zeros((ns, ncls + 1), dtype=bf)
    oh[np.arange(ns), labels] = 1
    oh[:, ncls] = 1  # ones column -> softmax denominator
    # oh[p, c, h] = onehot[c*128+p, h]
    ohh = np.ascontiguousarray(
        oh.reshape(ns // 128, 128, ncls + 1).transpose(1, 0, 2))
    nq_shard = query.shape[0] // n_cores
    ns_shard = ns // n_cores
    qh_all = _x_layout(query, kch)  # [NQ/512, 128, KCH, 512]
    nbq = nq_shard // 512
    in_maps = []
    for i in range(n_cores):
        sup_i = support[i * ns_shard:(i + 1) * ns_shard]
        in_maps.append({
            "supX": _x_layout(sup_i, kch, 256),
            "qX": np.ascontiguousarray(qh_all[i * nbq:(i + 1) * nbq]),
            "W": Wh,
            "b": bh,
            "onehot": ohh,
        })
    return in_maps


def _run(support, query, W, b, support_labels, num_classes, trace=False):
    ncls = int(num_classes)
    key = (support.shape[0], query.shape[0] // N_CORES, support.shape[1],
           W.shape[1], ncls + 1)
    nc = _get_nc(key)
    in_maps = _prep_inputs(support, query, W, b, support_labels, ncls, N_CORES)
    res = run_bass_kernel_spmd(nc, in_maps, list(range(N_CORES)), trace=trace)
    out = np.concatenate([r["out"] for r in res.results], axis=0)
    return out.astype(np.float32), res


def kernel(support, query, W, b, support_labels, num_classes):
    out, _ = _run(support, query, W, b, support_labels, num_classes, trace=False)
    return out
